# revision 7
# baseline (speedup 1.0000x reference)
"""Multi-head dot-product attention on 8 Trainium2 NeuronCores.

Sharding: 8 cores = 4 batches x 2 head-groups (8 heads each).
Each core computes its batch's QKV projections (its 8 heads), full
softmax attention for those heads, and a partial output projection.
The host sums the two head-group partials per batch and adds the
(linear) bo / bv contributions.

Per-core pipeline (all matmuls float32r = full-rate fp22, except P*V
which runs in bf16):
  A: Q^T,K^T [hd, L] and V [L, hd] projections from host-transposed X^T
  B: per (head, 1024-wide q-chunk): S^T tiles on PE, exp on ScalarE
     (bf16 out, scale=1/8 folded, max-subtraction skipped - scores are
     O(5) so exp is safe), P.V with a ones-column appended to V giving
     softmax denominators for free, normalization via a K=1 ones-outer-
     product broadcast matmul + vector multiply
  C: output projection (partial - contracts this core's 8 heads)

Shapes (hardcoded): B=4, L=2048, D=1024, H=16, Hd=64.
"""

import os
import sys

for _p in ("/opt/trn_rl_repo", os.path.expanduser("~/.axon_site/_ro/trn_rl_repo")):
    if os.path.isdir(_p) and _p not in sys.path:
        sys.path.insert(0, _p)

from contextlib import ExitStack

import ml_dtypes
import numpy as np

import concourse.bass as bass
import concourse.tile as tile
from concourse import bacc, mybir
from concourse.bass_utils import run_bass_kernel_spmd

F32 = mybir.dt.float32
F32R = mybir.dt.float32r
BF16 = mybir.dt.bfloat16

B, L, D, H, Hd = 4, 2048, 1024, 16, 64
HG = H // 2  # heads per core (head group)
HDG = HG * Hd  # 512: per-core projected width
KT = L // 128  # 16 k/l tiles
MT = HDG // 128  # 4 hd tiles
EXP_SCALE = 1.0 / np.sqrt(Hd)


def build_program(loop_n=1):
    nc = bacc.Bacc()

    xt_d = nc.dram_tensor("xt", [D, L], F32R, kind="ExternalInput")
    wq_d = nc.dram_tensor("wq", [D, HDG], F32R, kind="ExternalInput")
    wk_d = nc.dram_tensor("wk", [D, HDG], F32R, kind="ExternalInput")
    wv_d = nc.dram_tensor("wv", [D, HDG], F32R, kind="ExternalInput")
    wo_d = nc.dram_tensor("wo", [HDG, D], F32R, kind="ExternalInput")
    bq_d = nc.dram_tensor("bq", [HDG], F32, kind="ExternalInput")
    bk_d = nc.dram_tensor("bk", [HDG], F32, kind="ExternalInput")
    onesv_d = nc.dram_tensor("onesv", [128, KT, HG], BF16, kind="ExternalInput")
    ones1_d = nc.dram_tensor("ones1", [1, Hd], F32R, kind="ExternalInput")
    y_d = nc.dram_tensor("y", [L, D], F32, kind="ExternalOutput")

    with tile.TileContext(nc) as tc, \
            nc.allow_low_precision(reason="fp22/bf16 attention internals are intentional"):
        if loop_n == 1:
            with ExitStack() as ctx:
                kernel_body(ctx, tc, xt_d, wq_d, wk_d, wv_d, wo_d, bq_d, bk_d,
                            onesv_d, ones1_d, y_d)
        else:
            with tc.For_i(0, loop_n, 1):
                with ExitStack() as ctx:
                    kernel_body(ctx, tc, xt_d, wq_d, wk_d, wv_d, wo_d, bq_d,
                                bk_d, onesv_d, ones1_d, y_d)
    nc.compile()
    return nc


def kernel_body(ctx, tc, xt_d, wq_d, wk_d, wv_d, wo_d, bq_d, bk_d,
                onesv_d, ones1_d, y_d):
    nc = tc.nc
    Exp = mybir.ActivationFunctionType.Exp

    persist = ctx.enter_context(tc.tile_pool(name="persist", bufs=1))

    # persistent tensors
    qt_sb = persist.tile([128, MT, L], F32R)   # Q^T: [hd-in-tile, m, l]
    kt_sb = persist.tile([128, MT, L], F32R)   # K^T
    v_sb = persist.tile([128, KT, HG * (Hd + 1)], BF16)  # V + ones col per head
    bq_sb = persist.tile([128, MT], F32)
    bk_sb = persist.tile([128, MT], F32)
    ones1_sb = persist.tile([1, Hd], F32R)

    nc.sync.dma_start(bq_sb[:], bq_d.ap().rearrange("(m p) -> p m", p=128))
    nc.sync.dma_start(bk_sb[:], bk_d.ap().rearrange("(m p) -> p m", p=128))
    nc.sync.dma_start(ones1_sb[:], ones1_d.ap())
    # ones columns of V (col Hd of each head's 65-wide block)
    vv = v_sb[:].rearrange("p t (h c) -> p t h c", h=HG)
    nc.sync.dma_start(vv[:, :, :, Hd : Hd + 1].rearrange("p t h c -> p t (h c)"),
                      onesv_d.ap())

    # ---------------- Phase A: QKV projections ----------------
    with tc.tile_pool(name="xtp", bufs=1) as xtp, \
         tc.tile_pool(name="wvp", bufs=1) as wvp, \
         tc.tile_pool(name="wst", bufs=4) as wst, \
         tc.tile_pool(name="ppa", bufs=6, space="PSUM") as ppa:
        xt_sb = xtp.tile([128, D // 128, L], F32R)
        nc.sync.dma_start(xt_sb[:], xt_d.ap().rearrange("(k p) n -> p k n", p=128))

        # Q^T, K^T: out[m-tile] = W[:, m-tile].T @ X^T, chunks of 512 over l
        for w_d, dst_sb, bias_sb in ((wq_d, qt_sb, bq_sb), (wk_d, kt_sb, bk_sb)):
            for m in range(MT):
                psums = []
                for lc in range(4):
                    psums.append(ppa.tile([128, 512], F32, tag="pp", name=f"pp{lc}"))
                for k in range(D // 128):
                    wt = wst.tile([128, 128], F32R, tag="wt")
                    nc.sync.dma_start(
                        wt[:], w_d.ap()[k * 128 : (k + 1) * 128,
                                        m * 128 : (m + 1) * 128])
                    for lc in range(4):
                        nc.tensor.matmul(
                            psums[lc][:],
                            wt[:],
                            xt_sb[:, k, lc * 512 : (lc + 1) * 512],
                            start=(k == 0),
                            stop=(k == D // 128 - 1),
                        )
                for lc in range(4):
                    nc.vector.tensor_scalar_add(
                        dst_sb[:, m, lc * 512 : (lc + 1) * 512],
                        psums[lc][:],
                        bias_sb[:, m : m + 1],
                    )

        # V in natural layout: V[l-tile] = X^T[:, l-tile].T @ Wv
        wv_sb = wvp.tile([128, D // 128, HDG], F32R)
        nc.sync.dma_start(wv_sb[:], wv_d.ap().rearrange("(k p) n -> p k n", p=128))
        for lt in range(KT):
            ps_v = ppa.tile([128, 512], F32, tag="pp")
            for k in range(D // 128):
                nc.tensor.matmul(
                    ps_v[:],
                    xt_sb[:, k, lt * 128 : (lt + 1) * 128],
                    wv_sb[:, k, :],
                    start=(k == 0),
                    stop=(k == D // 128 - 1),
                )
            nc.vector.tensor_copy(
                vv[:, lt, :, 0:Hd],
                ps_v[:].rearrange("p (h c) -> p h c", h=HG),
            )

    # ---------------- Phase B: attention ----------------
    with tc.tile_pool(name="otn", bufs=1) as otnp:
      otn_sb = otnp.tile([128, MT, L], F32R)  # normalized O^T
      with tc.tile_pool(name="pb", bufs=4) as pb, \
           tc.tile_pool(name="nrm", bufs=3) as nrm, \
           tc.tile_pool(name="psb", bufs=2, space="PSUM") as psb, \
           tc.tile_pool(name="pob", bufs=1, space="PSUM") as pob, \
           tc.tile_pool(name="prb", bufs=1, space="PSUM") as prb:

        for h in range(HG):
            m_h = h // 2
            r_h = (h % 2) * 64
            for qc in range(2):
                q_sl = slice(qc * 1024, (qc + 1) * 1024)
                pts = []
                for t in range(KT):
                    ps_s = psb.tile([128, 1024], F32, tag="ps")
                    for half in range(2):
                        nc.tensor.matmul(
                            ps_s[:, half * 512 : (half + 1) * 512],
                            kt_sb[r_h : r_h + 64, m_h, t * 128 : (t + 1) * 128],
                            qt_sb[r_h : r_h + 64, m_h,
                                  qc * 1024 + half * 512 :
                                  qc * 1024 + (half + 1) * 512],
                            start=True,
                            stop=True,
                        )
                    pt = pb.tile([128, 1024], BF16, tag="pt", name="pt")
                    nc.scalar.activation(pt[:], ps_s[:], Exp, scale=EXP_SCALE)
                    pts.append(pt)
                ps_o = pob.tile([65, 1024], F32, tag="po")
                for t in range(KT):
                    for half in range(2):
                        nc.tensor.matmul(
                            ps_o[:, half * 512 : (half + 1) * 512],
                            v_sb[:, t, h * (Hd + 1) : (h + 1) * (Hd + 1)],
                            pts[t][:, half * 512 : (half + 1) * 512],
                            start=(t == 0),
                            stop=(t == KT - 1),
                        )
                # normalize: rows 0..63 /= row 64
                ot_u = nrm.tile([65, 1024], F32, tag="otu")
                nc.vector.tensor_copy(ot_u[:], ps_o[:])
                rc = nrm.tile([1, 1024], F32R, tag="rc")
                nc.vector.reciprocal(rc[:], ot_u[64:65, :])
                ps_r = prb.tile([64, 1024], F32, tag="pr")
                for half in range(2):
                    nc.tensor.matmul(
                        ps_r[:, half * 512 : (half + 1) * 512],
                        ones1_sb[:],
                        rc[:, half * 512 : (half + 1) * 512],
                        start=True, stop=True)
                nc.vector.tensor_tensor(
                    otn_sb[r_h : r_h + 64, m_h, q_sl],
                    ot_u[0:64, :],
                    ps_r[:],
                    op=mybir.AluOpType.mult,
                )

      # ---------------- Phase C: output projection ----------------
      with tc.tile_pool(name="wop", bufs=1) as wop, \
           tc.tile_pool(name="yst", bufs=3) as yst, \
           tc.tile_pool(name="pyc", bufs=2, space="PSUM") as pyc:
          wo_sb = wop.tile([128, MT, D], F32R)
          nc.sync.dma_start(wo_sb[:], wo_d.ap().rearrange("(k p) n -> p k n", p=128))
          for mq in range(KT):
              for nch in range(2):
                  n_sl = slice(nch * 512, (nch + 1) * 512)
                  ps_y = pyc.tile([128, 512], F32, tag="py")
                  for k in range(MT):
                      nc.tensor.matmul(
                          ps_y[:],
                          otn_sb[:, k, mq * 128 : (mq + 1) * 128],
                          wo_sb[:, k, n_sl],
                          start=(k == 0),
                          stop=(k == MT - 1),
                      )
                  yt = yst.tile([128, 512], F32, tag="yt")
                  nc.vector.tensor_copy(yt[:], ps_y[:])
                  nc.sync.dma_start(
                      y_d.ap()[mq * 128 : (mq + 1) * 128, n_sl], yt[:])


_PROGRAM_CACHE = {}


def _get_program():
    if "nc" not in _PROGRAM_CACHE:
        _PROGRAM_CACHE["nc"] = build_program()
    return _PROGRAM_CACHE["nc"]


def make_in_maps(inputs):
    x = np.asarray(inputs["x"], dtype=np.float32)
    wq = np.asarray(inputs["wq"], dtype=np.float32)
    wk = np.asarray(inputs["wk"], dtype=np.float32)
    wv = np.asarray(inputs["wv"], dtype=np.float32)
    wo = np.asarray(inputs["wo"], dtype=np.float32)
    bq = np.asarray(inputs["bq"], dtype=np.float32)
    bk = np.asarray(inputs["bk"], dtype=np.float32)

    onesv = np.ones((128, KT, HG), dtype=ml_dtypes.bfloat16)
    ones1 = np.ones((1, Hd), dtype=np.float32)

    in_maps = []
    for c in range(8):
        b, g = divmod(c, 2)
        hs = slice(g * HG, (g + 1) * HG)
        in_maps.append({
            "xt": np.ascontiguousarray(x[b].T),
            "wq": np.ascontiguousarray(wq[:, hs, :]).reshape(D, HDG),
            "wk": np.ascontiguousarray(wk[:, hs, :]).reshape(D, HDG),
            "wv": np.ascontiguousarray(wv[:, hs, :]).reshape(D, HDG),
            "wo": np.ascontiguousarray(wo[hs]).reshape(HDG, D),
            "bq": np.ascontiguousarray(bq[hs]).reshape(HDG),
            "bk": np.ascontiguousarray(bk[hs]).reshape(HDG),
            "onesv": onesv,
            "ones1": ones1,
        })
    return in_maps


def kernel(x, wq, bq, wk, bk, wv, bv, wo, bo, _timing=None):
    wo = np.asarray(wo, dtype=np.float32)
    bv = np.asarray(bv, dtype=np.float32)
    bo = np.asarray(bo, dtype=np.float32)

    nc = _get_program()
    in_maps = make_in_maps(
        {"x": x, "wq": wq, "wk": wk, "wv": wv, "wo": wo, "bq": bq, "bk": bk})

    res = run_bass_kernel_spmd(nc, in_maps, list(range(8)))
    if _timing is not None:
        _timing["exec_time_ns"] = res.exec_time_ns
        _timing["results"] = res

    # host-side unshard: sum the two head-group partials per batch,
    # add the linear bias contributions (bo + sum_h bv_h @ wo_h).
    bias_row = bo + np.einsum("hd,hdo->o", bv, wo)
    out = np.empty((B, L, D), dtype=np.float32)
    for b in range(B):
        out[b] = res.results[2 * b]["y"] + res.results[2 * b + 1]["y"] + bias_row
    return out


# revision 21
# speedup vs baseline: 1.4676x; 1.4676x over previous
"""Multi-head dot-product attention on 8 Trainium2 NeuronCores.

Sharding: 8 cores = 4 batches x 2 head-groups (8 heads each).
Each core computes its batch's QKV projections (its 8 heads), full
softmax attention for those heads, and a partial output projection.
The host sums the two head-group partials per batch and adds the
(linear) bo / bv contributions.

Per-core pipeline (all matmuls float32r = full-rate fp22, except P*V
which runs in bf16):
  A: Q^T,K^T [hd, L] and V [L, hd] projections from host-transposed X^T
  B: per (head, 1024-wide q-chunk): S^T tiles on PE, exp on ScalarE
     (bf16 out, scale=1/8 folded, max-subtraction skipped - scores are
     O(5) so exp is safe), P.V with a ones-column appended to V giving
     softmax denominators for free, normalization via a K=1 ones-outer-
     product broadcast matmul + vector multiply
  C: output projection (partial - contracts this core's 8 heads)

Shapes (hardcoded): B=4, L=2048, D=1024, H=16, Hd=64.
"""

import os
import sys

for _p in ("/opt/trn_rl_repo", os.path.expanduser("~/.axon_site/_ro/trn_rl_repo")):
    if os.path.isdir(_p) and _p not in sys.path:
        sys.path.insert(0, _p)

from contextlib import ExitStack

import ml_dtypes
import numpy as np

import concourse.bass as bass
import concourse.tile as tile
from concourse import bacc, mybir
from concourse.bass_utils import run_bass_kernel_spmd

F32 = mybir.dt.float32
F32R = mybir.dt.float32r
BF16 = mybir.dt.bfloat16

B, L, D, H, Hd = 4, 2048, 1024, 16, 64
HG = H // 2  # heads per core (head group)
HDG = HG * Hd  # 512: per-core projected width
KT = L // 128  # 16 k/l tiles
MT = HDG // 128  # 4 hd tiles
EXP_SCALE = 1.0 / np.sqrt(Hd)


def build_program(loop_n=1, phases='ABC'):
    nc = bacc.Bacc()

    xt_d = nc.dram_tensor("xt", [D, L], BF16, kind="ExternalInput")
    wq_d = nc.dram_tensor("wq", [D, HDG], BF16, kind="ExternalInput")
    wk_d = nc.dram_tensor("wk", [D, HDG], BF16, kind="ExternalInput")
    wv_d = nc.dram_tensor("wv", [D, HDG], BF16, kind="ExternalInput")
    wo_d = nc.dram_tensor("wo", [HDG, D], BF16, kind="ExternalInput")
    bq_d = nc.dram_tensor("bq", [HDG], F32, kind="ExternalInput")
    bk_d = nc.dram_tensor("bk", [HDG], F32, kind="ExternalInput")
    onesv_d = nc.dram_tensor("onesv", [128, KT, HG], BF16, kind="ExternalInput")
    recip_d = nc.dram_tensor("recip_scratch", [64, 1024], F32)
    y_d = nc.dram_tensor("y", [L, D], F32, kind="ExternalOutput")

    with tile.TileContext(nc) as tc, \
            nc.allow_low_precision(reason="fp22/bf16 attention internals are intentional"):
        if loop_n == 1:
            with ExitStack() as ctx:
                kernel_body(ctx, tc, xt_d, wq_d, wk_d, wv_d, wo_d, bq_d, bk_d,
                            onesv_d, recip_d, y_d, phases)
        else:
            with tc.For_i(0, loop_n, 1):
                with ExitStack() as ctx:
                    kernel_body(ctx, tc, xt_d, wq_d, wk_d, wv_d, wo_d, bq_d,
                                bk_d, onesv_d, recip_d, y_d, phases)
    nc.compile()
    return nc


def kernel_body(ctx, tc, xt_d, wq_d, wk_d, wv_d, wo_d, bq_d, bk_d,
                onesv_d, recip_d, y_d, phases="ABC"):
    nc = tc.nc
    Exp = mybir.ActivationFunctionType.Exp

    persist = ctx.enter_context(tc.tile_pool(name="persist", bufs=1))

    # persistent tensors
    qt_sb = persist.tile([128, MT, L], BF16)   # Q^T: [hd-in-tile, m, l]
    kt_sb = persist.tile([128, MT, L], BF16)   # K^T
    v_sb = persist.tile([128, KT, HG * (Hd + 1)], BF16)  # V + ones col per head
    bq_sb = persist.tile([128, MT], F32)
    bk_sb = persist.tile([128, MT], F32)
    sums_sb = persist.tile([64, 1024], F32)

    nc.sync.dma_start(bq_sb[:], bq_d.ap().rearrange("(m p) -> p m", p=128))
    nc.sync.dma_start(bk_sb[:], bk_d.ap().rearrange("(m p) -> p m", p=128))
    # ones columns of V (col Hd of each head's 65-wide block)
    vv = v_sb[:].rearrange("p t (h c) -> p t h c", h=HG)
    nc.sync.dma_start(vv[:, :, :, Hd : Hd + 1].rearrange("p t h c -> p t (h c)"),
                      onesv_d.ap())

    # ---------------- Phase A: QKV projections ----------------
    with tc.tile_pool(name="xtp", bufs=1) as xtp, \
         tc.tile_pool(name="wvp", bufs=1) as wvp, \
         tc.tile_pool(name="wst", bufs=4) as wst, \
         tc.tile_pool(name="ppa", bufs=6, space="PSUM") as ppa:
        xt_sb = xtp.tile([128, D // 128, L], BF16)
        xt_r = xt_d.ap().rearrange("(k p) n -> p k n", p=128)
        dmae = [nc.sync, nc.gpsimd]
        for k in range(D // 128):
            dmae[k % 2].dma_start(xt_sb[:, k, :], xt_r[:, k, :])

        # Q^T, K^T: out[m-tile] = W[:, m-tile].T @ X^T, chunks of 512 over l
        for w_d, dst_sb, bias_sb in ((wq_d, qt_sb, bq_sb), (wk_d, kt_sb, bk_sb)):
            w_r = w_d.ap().rearrange("(k p) m -> p k m", p=128)
            for m in range(MT):
                psums = []
                for lc in range(4):
                    psums.append(ppa.tile([128, 512], F32, tag="pp", name=f"pp{lc}"))
                wt = wst.tile([128, D // 128, 128], BF16, tag="wt")
                dmae[m % 2].dma_start(wt[:], w_r[:, :, m * 128 : (m + 1) * 128])
                for k in range(D // 128):
                    for lc in range(4):
                        nc.tensor.matmul(
                            psums[lc][:],
                            wt[:, k, :],
                            xt_sb[:, k, lc * 512 : (lc + 1) * 512],
                            start=(k == 0),
                            stop=(k == D // 128 - 1),
                        )
                for lc in range(4):
                    nc.scalar.activation(
                        dst_sb[:, m, lc * 512 : (lc + 1) * 512],
                        psums[lc][:],
                        mybir.ActivationFunctionType.Identity,
                        bias=bias_sb[:, m : m + 1],
                    )

        # V in natural layout: V[l-tile] = X^T[:, l-tile].T @ Wv
        wv_sb = wvp.tile([128, D // 128, HDG], BF16)
        wv_r = wv_d.ap().rearrange("(k p) n -> p k n", p=128)
        for k in range(D // 128):
            dmae[k % 2].dma_start(wv_sb[:, k, :], wv_r[:, k, :])
        for lt in range(KT):
            ps_v = ppa.tile([128, 512], F32, tag="pp")
            for k in range(D // 128):
                nc.tensor.matmul(
                    ps_v[:],
                    xt_sb[:, k, lt * 128 : (lt + 1) * 128],
                    wv_sb[:, k, :],
                    start=(k == 0),
                    stop=(k == D // 128 - 1),
                )
            nc.vector.tensor_copy(
                vv[:, lt, :, 0:Hd],
                ps_v[:].rearrange("p (h c) -> p h c", h=HG),
            )

    # ---------------- Phase B: attention ----------------
    if "B" not in phases:
        return
    with tc.tile_pool(name="otn", bufs=1) as otnp:
      otn_sb = otnp.tile([128, MT, L], BF16)  # normalized O^T
      with tc.tile_pool(name="pb", bufs=6) as pb, \
           tc.tile_pool(name="nrm", bufs=3) as nrm, \
           tc.tile_pool(name="psb", bufs=3, space="PSUM") as psb, \
           tc.tile_pool(name="pob", bufs=1, space="PSUM") as pob:

        recip_sb = otnp.tile([64, 1024], F32)

        def normalize_batch(lo, hi):
            # softmax denominators for units [lo, hi): one batched reciprocal,
            # then DMA partition-broadcast + in-place multiply per unit
            b0 = (lo // 8) * 32
            nc.vector.reciprocal(recip_sb[b0 : b0 + 8, :],
                                 sums_sb[b0 : b0 + 8, :])
            nc.sync.dma_start(recip_d.ap()[b0 : b0 + 8, :],
                              recip_sb[b0 : b0 + 8, :])
            for u in range(lo, hi):
                h, qc = u // 2, u % 2
                j, i = h // 2, h % 2
                r = i * 64
                q_sl = slice(qc * 1024, (qc + 1) * 1024)
                ur = (u // 8) * 32 + u % 8
                rb = nrm.tile([128, 1024], F32, tag="rb")
                nc.gpsimd.dma_start(
                    rb[:], recip_d.ap()[ur : ur + 1, :].partition_broadcast(128))
                sl = otn_sb[r : r + 64, j, q_sl]
                nc.vector.tensor_tensor(sl, sl, rb[r : r + 64, :],
                                        op=mybir.AluOpType.mult)

        for h in range(HG):
            j = h // 2
            r = (h % 2) * 64
            for qc in range(2):
                q_sl = slice(qc * 1024, (qc + 1) * 1024)
                po = pob.tile([65, 1024], F32, tag="po", name="po")
                prev = None
                for t in range(KT + 1):
                    cur = None
                    if t < KT:
                        ps_s = psb.tile([128, 1024], F32, tag="ps", name="ps_s")
                        for half in range(2):
                            c0 = qc * 1024 + half * 512
                            nc.tensor.matmul(
                                ps_s[:, half * 512 : (half + 1) * 512],
                                kt_sb[r : r + 64, j, t * 128 : (t + 1) * 128],
                                qt_sb[r : r + 64, j, c0 : c0 + 512],
                                start=True,
                                stop=True,
                            )
                        cur = pb.tile([128, 1024], BF16, tag="pt", name="pt")
                        nc.scalar.activation(cur[:], ps_s[:], Exp,
                                             scale=EXP_SCALE)
                    if t > 0:
                        for half in range(2):
                            nc.tensor.matmul(
                                po[:, half * 512 : (half + 1) * 512],
                                v_sb[:, t - 1,
                                     h * (Hd + 1) : (h + 1) * (Hd + 1)],
                                prev[:, half * 512 : (half + 1) * 512],
                                start=(t == 1),
                                stop=(t == KT),
                            )
                    prev = cur
                # stage unnormalized O^T rows; sums row -> sums_sb[u]
                u = h * 2 + qc
                nc.vector.tensor_copy(otn_sb[r : r + 64, j, q_sl], po[0:64, :])
                srow = nrm.tile([65, 1024], F32, tag="sst", name="srow")
                nc.vector.tensor_copy(srow[64:65, :], po[64:65, :])
                ur = (u // 8) * 32 + u % 8
                nc.gpsimd.dma_start(sums_sb[ur : ur + 1, :], srow[64:65, :])
            if h == 3:
                normalize_batch(0, 8)
        normalize_batch(8, 16)


      # ---------------- Phase C: output projection ----------------
      if "C" not in phases:
          return
      with tc.tile_pool(name="wop", bufs=1) as wop, \
           tc.tile_pool(name="yst", bufs=3) as yst, \
           tc.tile_pool(name="pyc", bufs=2, space="PSUM") as pyc:
          wo_sb = wop.tile([128, MT, D], BF16)
          nc.sync.dma_start(wo_sb[:], wo_d.ap().rearrange("(k p) n -> p k n", p=128))
          for mq in range(KT):
              for nch in range(2):
                  n_sl = slice(nch * 512, (nch + 1) * 512)
                  ps_y = pyc.tile([128, 512], F32, tag="py")
                  for k in range(MT):
                      nc.tensor.matmul(
                          ps_y[:],
                          otn_sb[:, k, mq * 128 : (mq + 1) * 128],
                          wo_sb[:, k, n_sl],
                          start=(k == 0),
                          stop=(k == MT - 1),
                      )
                  yt = yst.tile([128, 512], F32, tag="yt")
                  nc.vector.tensor_copy(yt[:], ps_y[:])
                  nc.sync.dma_start(
                      y_d.ap()[mq * 128 : (mq + 1) * 128, n_sl], yt[:])


_PROGRAM_CACHE = {}


def _get_program():
    if "nc" not in _PROGRAM_CACHE:
        _PROGRAM_CACHE["nc"] = build_program()
    return _PROGRAM_CACHE["nc"]


def make_in_maps(inputs):
    x = np.asarray(inputs["x"], dtype=np.float32)
    wq = np.asarray(inputs["wq"], dtype=np.float32)
    wk = np.asarray(inputs["wk"], dtype=np.float32)
    wv = np.asarray(inputs["wv"], dtype=np.float32)
    wo = np.asarray(inputs["wo"], dtype=np.float32)
    bq = np.asarray(inputs["bq"], dtype=np.float32)
    bk = np.asarray(inputs["bk"], dtype=np.float32)

    onesv = np.ones((128, KT, HG), dtype=ml_dtypes.bfloat16)

    in_maps = []
    for c in range(8):
        b, g = divmod(c, 2)
        hs = slice(g * HG, (g + 1) * HG)
        in_maps.append({
            "xt": np.ascontiguousarray(x[b].T).astype(ml_dtypes.bfloat16),
            "wq": np.ascontiguousarray(wq[:, hs, :]).reshape(D, HDG).astype(ml_dtypes.bfloat16),
            "wk": np.ascontiguousarray(wk[:, hs, :]).reshape(D, HDG).astype(ml_dtypes.bfloat16),
            "wv": np.ascontiguousarray(wv[:, hs, :]).reshape(D, HDG).astype(ml_dtypes.bfloat16),
            "wo": np.ascontiguousarray(wo[hs]).reshape(HDG, D).astype(ml_dtypes.bfloat16),
            "bq": np.ascontiguousarray(bq[hs]).reshape(HDG),
            "bk": np.ascontiguousarray(bk[hs]).reshape(HDG),
            "onesv": onesv,
            })
    return in_maps


def kernel(x, wq, bq, wk, bk, wv, bv, wo, bo, _timing=None):
    wo = np.asarray(wo, dtype=np.float32)
    bv = np.asarray(bv, dtype=np.float32)
    bo = np.asarray(bo, dtype=np.float32)

    nc = _get_program()
    in_maps = make_in_maps(
        {"x": x, "wq": wq, "wk": wk, "wv": wv, "wo": wo, "bq": bq, "bk": bk})

    res = run_bass_kernel_spmd(nc, in_maps, list(range(8)))
    if _timing is not None:
        _timing["exec_time_ns"] = res.exec_time_ns
        _timing["results"] = res

    # host-side unshard: sum the two head-group partials per batch,
    # add the linear bias contributions (bo + sum_h bv_h @ wo_h).
    bias_row = bo + np.einsum("hd,hdo->o", bv, wo)
    out = np.empty((B, L, D), dtype=np.float32)
    for b in range(B):
        out[b] = res.results[2 * b]["y"] + res.results[2 * b + 1]["y"] + bias_row
    return out


# revision 30
# speedup vs baseline: 1.9017x; 1.2958x over previous
"""Multi-head dot-product attention on 8 Trainium2 NeuronCores.

Sharding: 8 cores = 4 batches x 2 head-groups (8 heads each).
Each core computes its batch's QKV projections (its 8 heads), full
softmax attention for those heads, and a partial output projection.
The host sums the two head-group partials per batch and adds the
(linear) bo / bv contributions.

Per-core pipeline (all matmuls bf16 with fp32 PSUM accumulation):
  A: Q^T,K^T [hd, L] and V [L, hd] projections from host-transposed X^T
  B: per (head, 1024-wide q-chunk): S^T tiles on PE (3 PSUM score bufs
     for cross-engine slack), exp on ScalarE (bf16 out, scale=1/8
     folded, max-subtraction skipped - scores are O(5) so exp is safe),
     P.V with a ones-column appended to V giving softmax denominators
     for free. Normalization: denominators gathered via DMA into one
     tile, two batched fp32 reciprocals (single-partition reciprocals
     are ~8.6us each - the iterative-divide ALU runs 8 cyc/elem on one
     lane), DMA partition-broadcast via DRAM scratch, in-place multiply
  C: output projection (partial - contracts this core's 8 heads)

Shapes (hardcoded): B=4, L=2048, D=1024, H=16, Hd=64.
"""

import os
import sys

for _p in ("/opt/trn_rl_repo", os.path.expanduser("~/.axon_site/_ro/trn_rl_repo")):
    if os.path.isdir(_p) and _p not in sys.path:
        sys.path.insert(0, _p)

from contextlib import ExitStack

import ml_dtypes
import numpy as np

import concourse.bass as bass
import concourse.tile as tile
from concourse import bacc, mybir
from concourse.bass_utils import run_bass_kernel_spmd

F32 = mybir.dt.float32
F32R = mybir.dt.float32r
BF16 = mybir.dt.bfloat16

B, L, D, H, Hd = 4, 2048, 1024, 16, 64
HG = H // 2  # heads per core (head group)
HDG = HG * Hd  # 512: per-core projected width
KT = L // 128  # 16 k/l tiles
MT = HDG // 128  # 4 hd tiles
EXP_SCALE = 1.0 / np.sqrt(Hd)


def build_program(loop_n=1, phases='ABC'):
    nc = bacc.Bacc()

    xt_d = nc.dram_tensor("xt", [D, L], BF16, kind="ExternalInput")
    wq_d = nc.dram_tensor("wq", [D, HDG], BF16, kind="ExternalInput")
    wk_d = nc.dram_tensor("wk", [D, HDG], BF16, kind="ExternalInput")
    wv_d = nc.dram_tensor("wv", [D, HDG], BF16, kind="ExternalInput")
    wo_d = nc.dram_tensor("wo", [HDG, D], BF16, kind="ExternalInput")
    bq_d = nc.dram_tensor("bq", [HDG], F32, kind="ExternalInput")
    bk_d = nc.dram_tensor("bk", [HDG], F32, kind="ExternalInput")
    onesv_d = nc.dram_tensor("onesv", [128, KT, HG], BF16, kind="ExternalInput")
    recip_d = nc.dram_tensor("recip_scratch", [128, 1024], F32)
    y_d = nc.dram_tensor("y", [L, D], F32, kind="ExternalOutput")

    with tile.TileContext(nc) as tc, \
            nc.allow_low_precision(reason="fp22/bf16 attention internals are intentional"):
        if loop_n == 1:
            with ExitStack() as ctx:
                kernel_body(ctx, tc, xt_d, wq_d, wk_d, wv_d, wo_d, bq_d, bk_d,
                            onesv_d, recip_d, y_d, phases)
        else:
            with tc.For_i(0, loop_n, 1):
                with ExitStack() as ctx:
                    kernel_body(ctx, tc, xt_d, wq_d, wk_d, wv_d, wo_d, bq_d,
                                bk_d, onesv_d, recip_d, y_d, phases)
    nc.compile()
    return nc


def kernel_body(ctx, tc, xt_d, wq_d, wk_d, wv_d, wo_d, bq_d, bk_d,
                onesv_d, recip_d, y_d, phases="ABC"):
    nc = tc.nc
    Exp = mybir.ActivationFunctionType.Exp

    persist = ctx.enter_context(tc.tile_pool(name="persist", bufs=1))

    # persistent tensors
    qt_sb = persist.tile([128, MT, L], BF16)   # Q^T: [hd-in-tile, m, l]
    kt_sb = persist.tile([128, MT, L], BF16)   # K^T
    v_sb = persist.tile([128, KT, HG * (Hd + 1)], BF16)  # V + ones col per head
    bq_sb = persist.tile([128, MT], F32)
    bk_sb = persist.tile([128, MT], F32)
    sums_sb = persist.tile([128, 1024], F32)

    nc.sync.dma_start(bq_sb[:], bq_d.ap().rearrange("(m p) -> p m", p=128))
    nc.sync.dma_start(bk_sb[:], bk_d.ap().rearrange("(m p) -> p m", p=128))
    # ones columns of V (col Hd of each head's 65-wide block)
    vv = v_sb[:].rearrange("p t (h c) -> p t h c", h=HG)
    nc.sync.dma_start(vv[:, :, :, Hd : Hd + 1].rearrange("p t h c -> p t (h c)"),
                      onesv_d.ap())

    # ---------------- Phase A: QKV projections ----------------
    with tc.tile_pool(name="xtp", bufs=1) as xtp, \
         tc.tile_pool(name="wvp", bufs=1) as wvp, \
         tc.tile_pool(name="wst", bufs=6) as wst, \
         tc.tile_pool(name="ppa", bufs=8, space="PSUM") as ppa:
        xt_sb = xtp.tile([128, D // 128, L], BF16)
        xt_r = xt_d.ap().rearrange("(k p) n -> p k n", p=128)
        dmae = [nc.sync, nc.gpsimd]
        for k in range(D // 128):
            dmae[k % 2].dma_start(xt_sb[:, k, :], xt_r[:, k, :])

        # Q^T, K^T: out[m-tile] = W[:, m-tile].T @ X^T, chunks of 512 over l.
        # m-tile outer / tensor inner so kt[0] (which gates phase B's first
        # scores) is ready after the first m pass, not after all of Q.
        wq_r = wq_d.ap().rearrange("(k p) m -> p k m", p=128)
        wk_r = wk_d.ap().rearrange("(k p) m -> p k m", p=128)
        for m in range(MT):
            for w_r, dst_sb, bias_sb in ((wq_r, qt_sb, bq_sb),
                                         (wk_r, kt_sb, bk_sb)):
                psums = []
                for lc in range(4):
                    psums.append(ppa.tile([128, 512], F32, tag="pp", name=f"pp{lc}"))
                wt = wst.tile([128, D // 128, 128], BF16, tag="wt")
                dmae[m % 2].dma_start(wt[:], w_r[:, :, m * 128 : (m + 1) * 128])
                for k in range(D // 128):
                    for lc in range(4):
                        nc.tensor.matmul(
                            psums[lc][:],
                            wt[:, k, :],
                            xt_sb[:, k, lc * 512 : (lc + 1) * 512],
                            start=(k == 0),
                            stop=(k == D // 128 - 1),
                        )
                for lc in range(4):
                    nc.scalar.activation(
                        dst_sb[:, m, lc * 512 : (lc + 1) * 512],
                        psums[lc][:],
                        mybir.ActivationFunctionType.Identity,
                        bias=bias_sb[:, m : m + 1],
                    )

        # V in natural layout: V[l-tile] = X^T[:, l-tile].T @ Wv
        wv_sb = wvp.tile([128, D // 128, HDG], BF16)
        wv_r = wv_d.ap().rearrange("(k p) n -> p k n", p=128)
        for k in range(D // 128):
            dmae[k % 2].dma_start(wv_sb[:, k, :], wv_r[:, k, :])
        for lt in range(KT):
            ps_v = ppa.tile([128, 512], F32, tag="pp")
            for k in range(D // 128):
                nc.tensor.matmul(
                    ps_v[:],
                    xt_sb[:, k, lt * 128 : (lt + 1) * 128],
                    wv_sb[:, k, :],
                    start=(k == 0),
                    stop=(k == D // 128 - 1),
                )
            nc.vector.tensor_copy(
                vv[:, lt, :, 0:Hd],
                ps_v[:].rearrange("p (h c) -> p h c", h=HG),
            )

    # ---------------- Phase B: attention ----------------
    if "B" not in phases:
        return
    with tc.tile_pool(name="otn", bufs=1) as otnp:
      otn_sb = otnp.tile([128, MT, L], BF16)  # normalized O^T
      with tc.tile_pool(name="pb", bufs=8) as pb, \
           tc.tile_pool(name="nrm", bufs=4) as nrm, \
           tc.tile_pool(name="psb", bufs=3, space="PSUM") as psb, \
           tc.tile_pool(name="pob", bufs=1, space="PSUM") as pob:

        recip_sb = otnp.tile([128, 1024], F32)

        def normalize_batch(lo, hi):
            # softmax denominators for units [lo, hi): one batched reciprocal,
            # then DMA partition-broadcast + in-place multiply per unit
            n = hi - lo
            b0 = (lo // 4) * 32
            assert lo % 4 == 0 and n == 4
            nc.vector.reciprocal(recip_sb[b0 : b0 + n, :],
                                 sums_sb[b0 : b0 + n, :])
            nc.sync.dma_start(recip_d.ap()[b0 : b0 + n, :],
                              recip_sb[b0 : b0 + n, :])
            for u in range(lo, hi):
                h, qc = u // 2, u % 2
                j, i = h // 2, h % 2
                r = i * 64
                q_sl = slice(qc * 1024, (qc + 1) * 1024)
                ur = (u // 4) * 32 + u % 4
                rb = nrm.tile([128, 1024], F32, tag="rb")
                nc.gpsimd.dma_start(
                    rb[:], recip_d.ap()[ur : ur + 1, :].partition_broadcast(128))
                sl = otn_sb[r : r + 64, j, q_sl]
                nc.vector.tensor_tensor(sl, sl, rb[r : r + 64, :],
                                        op=mybir.AluOpType.mult)

        for h in range(HG):
            j = h // 2
            r = (h % 2) * 64
            for qc in range(2):
                q_sl = slice(qc * 1024, (qc + 1) * 1024)
                po = pob.tile([65, 1024], F32, tag="po", name="po")
                prev = None
                for t in range(KT + 1):
                    cur = None
                    if t < KT:
                        ps_s = psb.tile([128, 1024], F32, tag="ps", name="ps_s")
                        for half in range(2):
                            c0 = qc * 1024 + half * 512
                            nc.tensor.matmul(
                                ps_s[:, half * 512 : (half + 1) * 512],
                                kt_sb[r : r + 64, j, t * 128 : (t + 1) * 128],
                                qt_sb[r : r + 64, j, c0 : c0 + 512],
                                start=True,
                                stop=True,
                            )
                        cur = pb.tile([128, 1024], BF16, tag="pt", name="pt")
                        nc.scalar.activation(cur[:], ps_s[:], Exp,
                                             scale=EXP_SCALE)
                    if t > 0:
                        for half in range(2):
                            nc.tensor.matmul(
                                po[:, half * 512 : (half + 1) * 512],
                                v_sb[:, t - 1,
                                     h * (Hd + 1) : (h + 1) * (Hd + 1)],
                                prev[:, half * 512 : (half + 1) * 512],
                                start=(t == 1),
                                stop=(t == KT),
                            )
                    prev = cur
                # stage unnormalized O^T rows; sums row -> sums_sb[u]
                u = h * 2 + qc
                nc.vector.tensor_copy(otn_sb[r : r + 64, j, q_sl], po[0:64, :])
                srow = nrm.tile([65, 1024], F32, tag="sst", name="srow")
                nc.vector.tensor_copy(srow[64:65, :], po[64:65, :])
                ur = (u // 4) * 32 + u % 4
                nc.gpsimd.dma_start(sums_sb[ur : ur + 1, :], srow[64:65, :])
            if h % 2 == 1 and h < HG - 1:
                normalize_batch((h - 1) * 2, (h + 1) * 2)
        normalize_batch(12, 16)


      # ---------------- Phase C: output projection ----------------
      if "C" not in phases:
          return
      with tc.tile_pool(name="wop", bufs=1) as wop, \
           tc.tile_pool(name="yst", bufs=3) as yst, \
           tc.tile_pool(name="pyc", bufs=4, space="PSUM") as pyc:
          wo_sb = wop.tile([128, MT, D], BF16)
          nc.sync.dma_start(wo_sb[:], wo_d.ap().rearrange("(k p) n -> p k n", p=128))
          for mq in range(KT):
              for nch in range(2):
                  n_sl = slice(nch * 512, (nch + 1) * 512)
                  ps_y = pyc.tile([128, 512], F32, tag="py")
                  for k in range(MT):
                      nc.tensor.matmul(
                          ps_y[:],
                          otn_sb[:, k, mq * 128 : (mq + 1) * 128],
                          wo_sb[:, k, n_sl],
                          start=(k == 0),
                          stop=(k == MT - 1),
                      )
                  yt = yst.tile([128, 512], F32, tag="yt")
                  nc.vector.tensor_copy(yt[:], ps_y[:])
                  nc.sync.dma_start(
                      y_d.ap()[mq * 128 : (mq + 1) * 128, n_sl], yt[:])


_PROGRAM_CACHE = {}


def _get_program():
    if "nc" not in _PROGRAM_CACHE:
        _PROGRAM_CACHE["nc"] = build_program()
    return _PROGRAM_CACHE["nc"]


def make_in_maps(inputs):
    x = np.asarray(inputs["x"], dtype=np.float32)
    wq = np.asarray(inputs["wq"], dtype=np.float32)
    wk = np.asarray(inputs["wk"], dtype=np.float32)
    wv = np.asarray(inputs["wv"], dtype=np.float32)
    wo = np.asarray(inputs["wo"], dtype=np.float32)
    bq = np.asarray(inputs["bq"], dtype=np.float32)
    bk = np.asarray(inputs["bk"], dtype=np.float32)

    onesv = np.ones((128, KT, HG), dtype=ml_dtypes.bfloat16)

    in_maps = []
    for c in range(8):
        b, g = divmod(c, 2)
        hs = slice(g * HG, (g + 1) * HG)
        in_maps.append({
            "xt": np.ascontiguousarray(x[b].T).astype(ml_dtypes.bfloat16),
            "wq": np.ascontiguousarray(wq[:, hs, :]).reshape(D, HDG).astype(ml_dtypes.bfloat16),
            "wk": np.ascontiguousarray(wk[:, hs, :]).reshape(D, HDG).astype(ml_dtypes.bfloat16),
            "wv": np.ascontiguousarray(wv[:, hs, :]).reshape(D, HDG).astype(ml_dtypes.bfloat16),
            "wo": np.ascontiguousarray(wo[hs]).reshape(HDG, D).astype(ml_dtypes.bfloat16),
            "bq": np.ascontiguousarray(bq[hs]).reshape(HDG),
            "bk": np.ascontiguousarray(bk[hs]).reshape(HDG),
            "onesv": onesv,
            })
    return in_maps


def kernel(x, wq, bq, wk, bk, wv, bv, wo, bo, _timing=None):
    wo = np.asarray(wo, dtype=np.float32)
    bv = np.asarray(bv, dtype=np.float32)
    bo = np.asarray(bo, dtype=np.float32)

    nc = _get_program()
    in_maps = make_in_maps(
        {"x": x, "wq": wq, "wk": wk, "wv": wv, "wo": wo, "bq": bq, "bk": bk})

    res = run_bass_kernel_spmd(nc, in_maps, list(range(8)))
    if _timing is not None:
        _timing["exec_time_ns"] = res.exec_time_ns
        _timing["results"] = res

    # host-side unshard: sum the two head-group partials per batch,
    # add the linear bias contributions (bo + sum_h bv_h @ wo_h).
    bias_row = bo + np.einsum("hd,hdo->o", bv, wo)
    out = np.empty((B, L, D), dtype=np.float32)
    for b in range(B):
        out[b] = res.results[2 * b]["y"] + res.results[2 * b + 1]["y"] + bias_row
    return out
